# revision 1
# baseline (speedup 1.0000x reference)
"""Positional-encoding broadcast kernel for Trainium2 (8 NeuronCores).

The reference builds the interleaved sin/cos PE table [4096, 2048] f32 and
broadcasts it to [32, 4096, 2048] -- a 1 GiB, purely memory-bound output.
Sharding: by sequence.  Core i owns rows [512*i, 512*(i+1)) (a 4 MiB PE
slice, computed on host bit-identically to the reference's f32 jax-on-CPU
math) and writes those rows for all 32 batches = 128 MiB of HBM writes per
core (~358 GB/s per-core roofline).

Device program (raw Bass; this walrus build allows only 1 sync-wait per
instruction, which rules out Tile's multi-lane DMA drain):
- SBUF layout r=2: tile[p, c*4096 + r*2048 + m] = pe[c*256 + 2p + r, m],
  so each broadcast-store descriptor covers 16 KiB contiguous DRAM.
- Chunk 0's load and its two 32 MiB broadcast stores (zero-stride batch
  dim) are chained back-to-back on the sync HWDGE ring: per-SDMA-engine
  FIFO orders the store reads after the load writes with no semaphore
  round-trip.  Chunk 1 loads concurrently on the scalar HWDGE ring; its
  stores follow after a (long-satisfied) sem wait.
Measured: 396 us HW exec, store stream at ~359 GB/s, output bit-exact.
"""

import numpy as np

B = 32
SEQ = 4096
D = 2048
N_CORES = 8
S_SHARD = SEQ // N_CORES          # 512
NCH = 2                           # chunks of 256 rows
R = 2                             # rows per partition
CW = R * D                        # 4096

_cache = {}


def _pe_table() -> np.ndarray:
    import jax
    import jax.numpy as jnp

    cpu = jax.devices("cpu")[0]
    with jax.default_device(cpu):
        n = 10000.0
        pos = jnp.arange(SEQ, dtype=jnp.float32)[:, None]
        i = jnp.arange(D // 2, dtype=jnp.float32)[None, :]
        theta = pos / jnp.power(n, (2.0 * i) / D)
        pe = jnp.stack([jnp.sin(theta), jnp.cos(theta)], axis=-1)
        pe = pe.reshape(SEQ, D)
        return np.asarray(jax.device_get(pe))


def build_nc():
    import concourse.bass as bass
    import concourse.mybir as mybir

    nc = bass.Bass()
    pe_in = nc.dram_tensor("pe", [S_SHARD, D], mybir.dt.float32, kind="ExternalInput")
    out = nc.dram_tensor(
        "out", [B, S_SHARD, D], mybir.dt.float32, kind="ExternalOutput"
    )
    with (
        nc.sbuf_tensor([128, NCH * CW], mybir.dt.float32) as tile,
        nc.semaphore("load_sem") as load_sem,
        nc.semaphore("store_sem") as store_sem,
        nc.Block() as block,
    ):
        pe_src = pe_in.rearrange("(c p r) m -> p c (r m)", c=NCH, p=128, r=R)
        tile_c = tile[:].rearrange("p (c x) -> p c x", c=NCH)

        def bcast_src(c):
            return (
                tile[:, c * CW : (c + 1) * CW]
                .unsqueeze(1)
                .broadcast_to([128, B // 2, CW])
            )

        def dst_view(c, h):
            return out[
                h * (B // 2) : (h + 1) * (B // 2),
                c * 256 : (c + 1) * 256,
                :,
            ].rearrange("b (p r) m -> p b (r m)", p=128, r=R)

        @block.scalar
        def _(scalar):
            scalar.dma_start(out=tile_c[:, 1, :], in_=pe_src[:, 1, :]).then_inc(
                load_sem, 16
            )

        @block.sync
        def _(sync):
            # Chunk 0: load + stores chained on one ring; per-engine FIFO
            # orders the store reads after the load writes, no sem wait.
            sync.dma_start(out=tile_c[:, 0, :], in_=pe_src[:, 0, :]).then_inc(
                load_sem, 16
            )
            for h in range(2):
                sync.dma_start(out=dst_view(0, h), in_=bcast_src(0)).then_inc(
                    store_sem, 16
                )
            # Chunk 1 loaded on the scalar ring; both loads inc load_sem.
            sync.wait_ge(load_sem, 32)
            for h in range(2):
                sync.dma_start(out=dst_view(1, h), in_=bcast_src(1)).then_inc(
                    store_sem, 16
                )
            sync.wait_ge(store_sem, 64)

    return nc


def make_in_maps(pe: np.ndarray):
    return [{"pe": pe[i * S_SHARD : (i + 1) * S_SHARD]} for i in range(N_CORES)]


def kernel(x: np.ndarray) -> np.ndarray:
    from concourse.bass_utils import run_bass_kernel_spmd

    assert x.shape[0] == B

    pe = _pe_table()
    if "nc" not in _cache:
        _cache["nc"] = build_nc()
    res = run_bass_kernel_spmd(_cache["nc"], make_in_maps(pe), list(range(N_CORES)))
    outs = [res.results[i]["out"] for i in range(N_CORES)]
    return np.concatenate(outs, axis=1)

